# revision 1
# baseline (speedup 1.0000x reference)
"""Causal multi-head self-attention (RoPE) Trainium2 Bass kernel.

Problem: x:(4,2048,1024), Wq/Wk/Wv:(1024,1024), Wo:(1024,1024), bo:(1024,)
  q,k,v = split_heads(x@W*), rope(q), rope(k), causal softmax(q k^T/8) v, @Wo+bo

Sharding: head-parallel across 8 cores. Core c owns heads {2c, 2c+1} for all
4 batches: it computes q/k/v projections against the 128-column weight slice,
attention for its heads, and a partial output projection against the matching
128-row slice of Wo. Host sums the 8 partial (8192,1024) outputs and adds bo.

On-core layout (all "T" tensors are feature-major: partitions=feature rows,
free=tokens):
  Q^T/K^T (128 x 2048/batch): rows = [h0 d-evens(32), h0 d-odds(32), h1 ...]
    (NeoX-style d-permutation, folded into the host-permuted weight columns;
     valid because q and k get the same permutation and qk^T is d-invariant)
  RoPE: Q <- Q*cos + (P2@Q)*sin2, where P2 swaps the even/odd halves per head
    (PE matmul) and sin2 carries the sign; 3 DVE passes per tensor-block.
  S^T tiles (tj x ti) = K^T.T @ Q^T per head (fp32r, K=64 contraction).
  A = exp(0.125*S^T) (ACT, straddle tiles band-masked with -1e30 triangle).
  O~^T (65 x ti) accumulated = [V|1].T @ A over tj chunks; row 64 = softmax
    denominators (ones column trick). Normalize via ACT reciprocal +
    DRAM-staged partition broadcast + DVE multiply -> O^T (128 x 2048).
  y partial (128t x 1024) = O^T-chunk.T @ Wo-slice, DMA'd psum->DRAM.
"""

import numpy as np

B, T, C = 4, 2048, 1024
H, D = 16, 64
N_CORES = 8
BT = B * T
SCALE = 0.125  # D**-0.5
NEG = -1.0e30

TRACE = False            # set True (e.g. from test.py) to capture an NTFF trace
LAST_RESULT = None       # BassKernelResults of the most recent run

_BUILT = None            # cached (nc, input-name list)


# --------------------------------------------------------------------------
# workaround: this walrus build rejects >1 semaphore wait per instruction
def _split_sem_waits(nc, max_waits=1):
    import concourse.mybir as mybir

    n = 0
    for f in nc.m.functions:
        for bb in f.blocks:
            insts = bb.instructions
            idx = 0
            while idx < len(insts):
                i = insts[idx]
                si = getattr(i, "sync_info", None)
                if si is not None and si.on_wait and len(si.on_wait) > max_waits:
                    waits = list(si.on_wait)
                    extra, keep = waits[:-max_waits], waits[-max_waits:]
                    si.on_wait = keep
                    pos = idx
                    for j in range(0, len(extra), max_waits):
                        n += 1
                        nd = mybir.InstNoOp(name=f"I-waitsplit-{n}", ins=[], outs=[])
                        nd.engine = i.engine
                        nd.sync_info = mybir.SyncInfo(
                            on_wait=extra[j : j + max_waits], on_update=[]
                        )
                        insts.insert(pos, nd)
                        pos += 1
                    idx = pos
                idx += 1


def _install_ntff_hook():
    """The image's antenv lacks axon_hooks; synthesize it so trace=True works."""
    import sys
    import types

    if "antenv.axon_hooks" in sys.modules:
        return
    import antenv

    state = {"hook": None}
    mod = types.ModuleType("antenv.axon_hooks")
    mod.get_axon_ntff_profile_hook = lambda: state["hook"]
    mod.set_axon_ntff_profile_hook = lambda h: state.__setitem__("hook", h)
    sys.modules["antenv.axon_hooks"] = mod
    antenv.axon_hooks = mod
    try:
        from trn_agent_boot.trn_boot import _ntff_profile_via_ctypes

        state["hook"] = _ntff_profile_via_ctypes("/opt/axon/libaxon_pjrt.so")
    except Exception:
        state["hook"] = None


# --------------------------------------------------------------------------
def _build():
    import concourse.bass as bass
    import concourse.mybir as mybir
    from concourse.tile import TileContext

    F = mybir.dt.float32
    MD = mybir.dt.float16  # matmul operand dtype
    MULT = mybir.AluOpType.mult
    ADD = mybir.AluOpType.add
    SUB = mybir.AluOpType.subtract
    EXP = mybir.ActivationFunctionType.Exp

    nc = bass.Bass()

    xT = nc.dram_tensor("xT", (C, BT), MD, kind="ExternalInput")
    wq = nc.dram_tensor("wq", (C, 128), MD, kind="ExternalInput")
    wk = nc.dram_tensor("wk", (C, 128), MD, kind="ExternalInput")
    wv = nc.dram_tensor("wv", (C, 128), MD, kind="ExternalInput")
    wo = nc.dram_tensor("wo", (128, C), MD, kind="ExternalInput")
    cosd = nc.dram_tensor("cos", (128, T), MD, kind="ExternalInput")
    sind = nc.dram_tensor("sin2", (128, T), MD, kind="ExternalInput")
    p2d = nc.dram_tensor("p2", (128, 128), MD, kind="ExternalInput")
    bandd = nc.dram_tensor("band2x", (128, 256), F, kind="ExternalInput")
    id2d = nc.dram_tensor("id2", (128, 64), F, kind="ExternalInput")
    vonesd = nc.dram_tensor("vones", (128, 32), MD, kind="ExternalInput")
    vzerod = nc.dram_tensor("vzero", (128, 1008), MD, kind="ExternalInput")
    y = nc.dram_tensor("y", (BT, C), F, kind="ExternalOutput")
    scr_s = nc.dram_tensor("scr_s", (B * 8, 512), F, kind="Internal")
    scr = nc.dram_tensor("scr", (B * 8, 512), F, kind="Internal")

    with TileContext(nc) as tc:
        with (
            tc.tile_pool(name="const", bufs=1) as cst,
            tc.tile_pool(name="xt", bufs=3) as xtp,
            tc.tile_pool(name="qt", bufs=2) as qp,
            tc.tile_pool(name="kt", bufs=2) as kp,
            tc.tile_pool(name="vt", bufs=2) as vp,
            tc.tile_pool(name="ot", bufs=2) as op_,
            tc.tile_pool(name="vst", bufs=2) as vstp,
            tc.tile_pool(name="tmp", bufs=4) as tmp,
            tc.tile_pool(name="at", bufs=6) as ap_,
            tc.tile_pool(name="bc", bufs=4) as bcp,
            tc.tile_pool(name="avs", bufs=4) as avsp,
            tc.tile_pool(name="rr", bufs=4) as rp,
            tc.tile_pool(name="ys", bufs=4) as ysp,
            tc.tile_pool(name="sps", bufs=2, space="PSUM") as sps,
            tc.tile_pool(name="stp", bufs=2, space="PSUM") as stp,
            tc.tile_pool(name="avp", bufs=2, space="PSUM") as avp,
        ):
            # ---- constants -------------------------------------------------
            wq_t = cst.tile([128, 8, 128], MD)
            wk_t = cst.tile([128, 8, 128], MD)
            wv_t = cst.tile([128, 8, 128], MD)
            for k in range(8):
                nc.sync.dma_start(out=wq_t[:, k, :], in_=wq[k * 128 : (k + 1) * 128, :])
                nc.sync.dma_start(out=wk_t[:, k, :], in_=wk[k * 128 : (k + 1) * 128, :])
                nc.sync.dma_start(out=wv_t[:, k, :], in_=wv[k * 128 : (k + 1) * 128, :])
            wo_t = cst.tile([128, C], MD)
            nc.sync.dma_start(out=wo_t, in_=wo[:, :])
            cos_t = cst.tile([128, T], MD)
            nc.sync.dma_start(out=cos_t, in_=cosd[:, :])
            sin_t = cst.tile([128, T], MD)
            nc.sync.dma_start(out=sin_t, in_=sind[:, :])
            p2_t = cst.tile([128, 128], MD)
            nc.sync.dma_start(out=p2_t, in_=p2d[:, :])
            band_t = cst.tile([128, 256], F)  # [band | band] for head pairs
            nc.sync.dma_start(out=band_t, in_=bandd[:, :])
            id_t = cst.tile([128, 64], F)
            nc.sync.dma_start(out=id_t, in_=id2d[:, :])

            QKV = {}  # b -> (Qb, Kb, Vb);  O = {} b -> Ob

            def phase_a_alloc(b):
                Qb = qp.tile([128, T], MD, name="Qb")
                Kb = kp.tile([128, T], MD, name="Kb")
                Vb = vp.tile([128, 16, 256], MD, name="Vb")  # per head 128 cols:
                # [d 0..63 | ones | zeros*63] so the AV lhsT is 128-wide (FWL)
                QKV[b] = (Qb, Kb, Vb)
                nc.sync.dma_start(
                    out=Vb[:, :, 64:256:128],
                    in_=vonesd[:, :].rearrange("p (a b) -> p a b", b=2),
                )
                zin = vzerod[:, :].rearrange("p (a b) -> p a b", b=63)
                nc.sync.dma_start(out=Vb[:, :, 65:128], in_=zin)
                nc.sync.dma_start(out=Vb[:, :, 193:256], in_=zin)

            def phase_a_unit(b, nb):
                Qb, Kb, Vb = QKV[b]
                if True:
                    g0 = b * T + nb * 512
                    cols = slice(nb * 512, (nb + 1) * 512)
                    xt = xtp.tile([128, 8, 512], MD, name="xt")
                    for k in range(8):
                        nc.sync.dma_start(
                            out=xt[:, k, :],
                            in_=xT[k * 128 : (k + 1) * 128, g0 : g0 + 512],
                        )
                    for W, dst in ((wq_t, Qb), (wk_t, Kb)):
                        ps = sps.tile([128, 512], F, tag="s", name="ps")
                        for k in range(8):
                            nc.tensor.matmul(
                                ps[:, :], lhsT=W[:, k, :], rhs=xt[:, k, :],
                                start=(k == 0), stop=(k == 7),
                            )
                        # rope: dst = qr*cos - P2@(qr*sin2)
                        #   (P2@ (q.sin2))[p] = -q~[p]*sin2[p], since sin2 is
                        #    antisymmetric and cos symmetric under the pair swap
                        qr = tmp.tile([128, 512], MD, name="qr")
                        nc.scalar.copy(qr[:, :], ps[:, :])
                        qs = tmp.tile([128, 512], MD, name="qs")
                        nc.vector.tensor_tensor(qs[:, :], qr[:, :],
                                                sin_t[:, cols], MULT)
                        nc.vector.tensor_tensor(dst[:, cols], qr[:, :],
                                                cos_t[:, cols], MULT)
                        rot = sps.tile([128, 512], F, tag="s", name="rot")
                        nc.tensor.matmul(rot[:, :], lhsT=p2_t[:, :], rhs=qs[:, :],
                                         start=True, stop=True)
                        nc.vector.tensor_tensor(dst[:, cols], dst[:, cols],
                                                rot[:, :], SUB)
                    ps = sps.tile([128, 512], F, tag="s", name="ps")
                    for k in range(8):
                        nc.tensor.matmul(
                            ps[:, :], lhsT=wv_t[:, k, :], rhs=xt[:, k, :],
                            start=(k == 0), stop=(k == 7),
                        )
                    vst = vstp.tile([128, 512], F, name="vst")
                    nc.scalar.copy(vst[:, :], ps[:, :])
                    for tl in range(4):
                        tt = nb * 4 + tl
                        tcs = slice(tl * 128, (tl + 1) * 128)
                        for h in (0, 1):
                            tp = sps.tile([128, 64], F, tag="s", name="tp")
                            nc.tensor.transpose(
                                tp[:, :], vst[64 * h : 64 * h + 64, tcs],
                                id_t[64 * h : 64 * h + 64, :],
                            )
                            nc.vector.tensor_copy(
                                Vb[:, tt, 128 * h : 128 * h + 64], tp[:, :])

            def y_unit(b, Ob, i):
                # output projection for the 4 token-tiles of ti-block i
                for tt in range(4 * i, 4 * i + 4):
                    lhs = Ob[:, tt * 128 : (tt + 1) * 128]
                    ysb = ysp.tile([128, 1024], F, name="ysb")
                    for nh in (0, 1):
                        yps = sps.tile([128, 512], F, tag="s", name="yps")
                        nc.tensor.matmul(
                            yps[:, :], lhsT=lhs,
                            rhs=wo_t[:, nh * 512 : (nh + 1) * 512],
                            start=True, stop=True,
                        )
                        if nh == 0:
                            nc.vector.tensor_copy(ysb[:, 0:512], yps[:, :])
                        else:
                            nc.scalar.copy(ysb[:, 512:1024], yps[:, :])
                    r0 = b * T + tt * 128
                    nc.sync.dma_start(out=y[r0 : r0 + 128, :], in_=ysb[:, :])

            def phase_d(b, filler=None, pre=None):
                Qb, Kb, Vb = QKV[b]
                Ob = op_.tile([128, T], MD, name="Ob")
                pending = []  # deferred y_units: keep normalize latency off
                # the PE critical path by emitting them a ti-block later
                for i in range(4):
                    if pre is not None:
                        pre(i)
                    av = [avp.tile([128, 512], F, tag="av", name="av")
                          for _ in (0, 1)]
                    nch = 4 * i + 4
                    sts = {}

                    def emit_st(j):
                        delta = j * 128 - i * 512
                        nl = 512 - max(0, delta)
                        off = 512 - nl
                        st = stp.tile([128, 2, 512], F, name="st")
                        for h in (0, 1):
                            hs = slice(64 * h, 64 * h + 64)
                            nc.tensor.matmul(
                                st[:, h, 0:nl],
                                lhsT=Kb[hs, j * 128 : (j + 1) * 128],
                                rhs=Qb[hs, i * 512 + off : (i + 1) * 512],
                                start=True, stop=True,
                            )
                        if delta >= 0:  # straddles the diagonal: mask triangle
                            nc.vector.tensor_tensor(
                                st[:, :, 0:128], st[:, :, 0:128],
                                band_t[:, :].rearrange("p (a c) -> p a c", a=2),
                                ADD)
                        sts[j] = (st, off, nl)

                    LAG = 1
                    for j in range(min(LAG, nch)):
                        emit_st(j)
                    for j in range(nch):
                        if j + LAG < nch:
                            emit_st(j + LAG)
                        if j == 1 and pending:
                            y_unit(b, Ob, pending.pop(0))
                        st, off, nl = sts.pop(j)
                        A = ap_.tile([128, 2, 512], MD, name="A")
                        nc.scalar.activation(
                            A[:, :, 0:nl], st[:, :, 0:nl], EXP, scale=SCALE)
                        for h in (0, 1):
                            nc.tensor.matmul(
                                av[h][0:128, off:512],
                                lhsT=Vb[:, j, 128 * h : 128 * h + 128],
                                rhs=A[:, h, 0:nl],
                                start=(j == 0), stop=(j == nch - 1),
                                skip_group_check=True,
                            )
                    for h in (0, 1):
                        row = b * 8 + i * 2 + h
                        # evacuate the accumulator to SBUF at once so the
                        # PSUM slot recycles without waiting on the
                        # reciprocal/broadcast DMA chain
                        avs = avsp.tile([65, 512], F, name="avs")
                        nc.vector.tensor_copy(avs[:, :], av[h][0:65, :])
                        # sums row -> DRAM -> (128x4) repartition -> lane-
                        # parallel reciprocal -> DRAM -> 64-row broadcast
                        srt = rp.tile([128, 4], F, name="srt")
                        nc.sync.dma_start(out=srt[:, :], in_=avs[64:65, :])
                        rt = rp.tile([128, 4], F, name="rt")
                        nc.vector.reciprocal(rt[:, :], srt[:, :])
                        nc.sync.dma_start(
                            out=scr[row : row + 1, :].rearrange(
                                "r (p c) -> (r p) c", c=4),
                            in_=rt[:, :],
                        )
                        bct = bcp.tile([64, 512], F, name="bct")
                        src = scr[row : row + 1, :]
                        bap = bass.AP(
                            tensor=src.tensor, offset=src.offset,
                            ap=[[0, 64]] + [list(p) for p in src.ap[1:]],
                        )
                        nc.sync.dma_start(out=bct[:, :], in_=bap)
                        nc.vector.tensor_tensor(
                            Ob[64 * h : 64 * h + 64, i * 512 : (i + 1) * 512],
                            avs[0:64, :], bct[:, :], MULT,
                        )
                    pending.append(i)
                    if filler is not None:
                        filler(i)
                for i2 in pending:
                    y_unit(b, Ob, i2)

            phase_a_alloc(0)
            for b in range(B):
                if b + 1 < B:
                    phase_a_alloc(b + 1)
                    fil = (lambda i, nb=b + 1: phase_a_unit(nb, i))
                else:
                    fil = None
                # batch 0's projection blocks are emitted just-in-time ahead
                # of the attention block that first needs them
                pre = (lambda i: phase_a_unit(0, i)) if b == 0 else None
                phase_d(b, filler=fil, pre=pre)

    _split_sem_waits(nc)
    return nc


# --------------------------------------------------------------------------
def _host_inputs(x, Wq, Wk, Wv):
    """Per-core input dicts (all shared arrays built once)."""
    BF = np.float16
    xT = np.ascontiguousarray(
        np.asarray(x, dtype=np.float32).reshape(BT, C).T).astype(BF)

    # NeoX d-permutation within each head: evens then odds
    dperm = np.concatenate([np.arange(0, D, 2), np.arange(1, D, 2)])

    inv_freq = (1.0 / (10000.0 ** (np.arange(0, D, 2) / D))).astype(np.float64)
    pos = np.arange(T, dtype=np.float64)
    ang = pos[None, :] * inv_freq[:, None]  # (32, T)
    cos32 = np.cos(ang).astype(np.float32)
    sin32 = np.sin(ang).astype(np.float32)
    cos_t = np.tile(np.vstack([cos32, cos32]), (2, 1))  # (128, T)
    sin_t = np.tile(np.vstack([-sin32, sin32]), (2, 1))  # (128, T), sign folded

    p2 = np.zeros((128, 128), dtype=np.float32)
    for hb in (0, 64):
        for i2 in range(32):
            p2[hb + i2, hb + 32 + i2] = 1.0
            p2[hb + 32 + i2, hb + i2] = 1.0

    band = np.where(
        np.arange(128)[None, :] < np.arange(128)[:, None], np.float32(NEG), 0.0
    ).astype(np.float32)
    band2x = np.concatenate([band, band], axis=1)  # (128, 256)
    id2 = np.tile(np.eye(D, dtype=np.float32), (2, 1))  # (128, 64)

    Wq = np.asarray(Wq, dtype=np.float32)
    Wk = np.asarray(Wk, dtype=np.float32)
    Wv = np.asarray(Wv, dtype=np.float32)

    in_maps = []
    for c in range(N_CORES):
        sl = slice(128 * c, 128 * (c + 1))
        wq_c = Wq[:, sl].reshape(C, 2, D)[:, :, dperm].reshape(C, 128)
        wk_c = Wk[:, sl].reshape(C, 2, D)[:, :, dperm].reshape(C, 128)
        in_maps.append({
            "xT": xT,
            "wq": np.ascontiguousarray(wq_c).astype(BF),
            "wk": np.ascontiguousarray(wk_c).astype(BF),
            "wv": np.ascontiguousarray(Wv[:, sl]).astype(BF),
            "wo": None,  # set below
            "cos": cos_t.astype(BF),
            "sin2": sin_t.astype(BF),
            "p2": p2.astype(BF),
            "band2x": band2x,
            "id2": id2,
            "vones": np.ones((128, 32), dtype=BF),
            "vzero": np.zeros((128, 1008), dtype=BF),
        })
    return in_maps


def kernel(x, Wq, Wk, Wv, Wo, bo):
    global _BUILT, LAST_RESULT
    from concourse.bass_utils import run_bass_kernel_spmd

    if TRACE:
        _install_ntff_hook()

    if _BUILT is None:
        _BUILT = _build()
    nc = _BUILT

    in_maps = _host_inputs(x, Wq, Wk, Wv)
    Wo = np.asarray(Wo, dtype=np.float32)
    for c in range(N_CORES):
        in_maps[c]["wo"] = np.ascontiguousarray(
            Wo[128 * c : 128 * (c + 1), :]).astype(np.float16)

    last_err = None
    for attempt in range(3):
        try:
            res = run_bass_kernel_spmd(
                nc, in_maps, core_ids=list(range(N_CORES)), trace=TRACE
            )
            break
        except Exception as e:  # transient NRT device errors: retry
            last_err = e
            import time as _time

            _time.sleep(2.0)
    else:
        raise last_err
    LAST_RESULT = res

    acc = res.results[0]["y"].astype(np.float64)
    for c in range(1, N_CORES):
        acc = acc + res.results[c]["y"]
    out = acc.astype(np.float32) + np.asarray(bo, dtype=np.float32)[None, :]
    return out.reshape(B, T, C)



# revision 18
# speedup vs baseline: 1.1744x; 1.1744x over previous
"""Causal multi-head self-attention (RoPE) Trainium2 Bass kernel (v2).

Problem: x:(4,2048,1024), Wq/Wk/Wv:(1024,1024), Wo:(1024,1024), bo:(1024,)
  q,k,v = split_heads(x@W*), rope(q), rope(k), causal softmax(q k^T/8) v, @Wo+bo

Sharding: head-parallel across 8 cores. Core c owns heads {2c, 2c+1} for all
4 batches: q/k/v projections against the 128-column weight slice, attention
for its heads, partial output projection against the matching 128-row slice
of Wo (pre-scaled by 1/16 to pair with the 16x-scaled softmax reciprocals).
Host sums the 8 partial (8192,1024) fp16 outputs and adds bo.

v2 changes vs v1 (all aimed at keeping the PE tensor engine continuously
busy so it holds its top p-state, and at getting latency chains off the
critical path):
  - Softmax normalize no longer round-trips DRAM: DVE reciprocal of the
    PSUM denominator row, 16x scale folded into Wo, fp16 K=2 PE matmul
    broadcasts the two heads' reciprocal rows across partitions.
  - Causal band mask is pre-written into the PSUM tile by the DVE before
    the QK^T matmul accumulates onto it (start=False), removing the
    mask -> exp dependency hop.
  - All independent PE work (projections, rope rotations, V transposes,
    output-projection tiles) is queued as fine-grained filler closures and
    pumped between the S-tile and AV matmuls of the attention inner loop.
  - Rope DVE ops read the projection PSUM directly (no scalar copy).
  - V path in fp16 end-to-end (cheaper transposes).
  - y partials in fp16; y DMAs alternate between the sync and gpsimd
    queues; x loads on sync; constants split across both queues.
"""

import numpy as np

B, T, C = 4, 2048, 1024
H, D = 16, 64
N_CORES = 8
BT = B * T
SCALE = 0.125  # D**-0.5
NEG = -1.0e30

TRACE = False            # set True (e.g. from test.py) to capture an NTFF trace
LAST_RESULT = None       # BassKernelResults of the most recent run

_BUILT = None            # cached nc


# --------------------------------------------------------------------------
# workaround: this walrus build rejects >1 semaphore wait per instruction
def _split_sem_waits(nc, max_waits=1):
    import concourse.mybir as mybir

    n = 0
    for f in nc.m.functions:
        for bb in f.blocks:
            insts = bb.instructions
            idx = 0
            while idx < len(insts):
                i = insts[idx]
                si = getattr(i, "sync_info", None)
                if si is not None and si.on_wait and len(si.on_wait) > max_waits:
                    waits = list(si.on_wait)
                    extra, keep = waits[:-max_waits], waits[-max_waits:]
                    si.on_wait = keep
                    pos = idx
                    for j in range(0, len(extra), max_waits):
                        n += 1
                        nd = mybir.InstNoOp(name=f"I-waitsplit-{n}", ins=[], outs=[])
                        nd.engine = i.engine
                        nd.sync_info = mybir.SyncInfo(
                            on_wait=extra[j : j + max_waits], on_update=[]
                        )
                        insts.insert(pos, nd)
                        pos += 1
                    idx = pos
                idx += 1


def _install_ntff_hook():
    """The image's antenv lacks axon_hooks; synthesize it so trace=True works."""
    import sys
    import types

    if "antenv.axon_hooks" in sys.modules:
        return
    import antenv

    state = {"hook": None}
    mod = types.ModuleType("antenv.axon_hooks")
    mod.get_axon_ntff_profile_hook = lambda: state["hook"]
    mod.set_axon_ntff_profile_hook = lambda h: state.__setitem__("hook", h)
    sys.modules["antenv.axon_hooks"] = mod
    antenv.axon_hooks = mod
    try:
        from trn_agent_boot.trn_boot import _ntff_profile_via_ctypes

        state["hook"] = _ntff_profile_via_ctypes("/opt/axon/libaxon_pjrt.so")
    except Exception:
        state["hook"] = None


# --------------------------------------------------------------------------
def _build():
    import concourse.bass as bass
    import concourse.mybir as mybir
    from concourse.tile import TileContext

    F = mybir.dt.float32
    MD = mybir.dt.float16  # matmul operand dtype
    MULT = mybir.AluOpType.mult
    SUB = mybir.AluOpType.subtract
    EXP = mybir.ActivationFunctionType.Exp

    nc = bass.Bass()

    xT = nc.dram_tensor("xT", (C, BT), MD, kind="ExternalInput")
    wq = nc.dram_tensor("wq", (C, 128), MD, kind="ExternalInput")
    wk = nc.dram_tensor("wk", (C, 128), MD, kind="ExternalInput")
    wv = nc.dram_tensor("wv", (C, 128), MD, kind="ExternalInput")
    wo = nc.dram_tensor("wo", (128, C), MD, kind="ExternalInput")  # pre /16
    cosd = nc.dram_tensor("cos", (128, T), MD, kind="ExternalInput")
    sind = nc.dram_tensor("sin2", (128, T), MD, kind="ExternalInput")
    p2d = nc.dram_tensor("p2", (128, 128), MD, kind="ExternalInput")
    bandTd = nc.dram_tensor("bandT2x", (128, 256), F, kind="ExternalInput")
    idfd = nc.dram_tensor("id128f", (128, 128), F, kind="ExternalInput")
    idhd = nc.dram_tensor("id128h", (128, 128), MD, kind="ExternalInput")
    seld = nc.dram_tensor("sel2", (2, 128), MD, kind="ExternalInput")
    vonesd = nc.dram_tensor("vones", (128, 32), MD, kind="ExternalInput")
    vzerod = nc.dram_tensor("vzero", (128, 1008), MD, kind="ExternalInput")
    y = nc.dram_tensor("y", (BT, C), MD, kind="ExternalOutput")

    from contextlib import ExitStack

    with TileContext(nc) as tc:
        with ExitStack() as _es:
            def _pool(name, bufs, space="SBUF"):
                return _es.enter_context(
                    tc.tile_pool(name=name, bufs=bufs, space=space))

            cst = _pool("const", 1)
            xtp = _pool("xt", 3)
            qp = _pool("qt", 2)
            kp = _pool("kt", 2)
            vp = _pool("vt", 2)
            op_ = _pool("ot", 2)
            vstp = _pool("vst", 2)
            qsp = _pool("qs", 2)
            evp = _pool("ev", 2)
            ap_ = _pool("at", 4)
            avsp = _pool("avs", 4)
            drtp = _pool("drt", 2)
            rcpp = _pool("rcp", 2)
            rc16p = _pool("rc16", 2)
            rrp = _pool("rrow", 4)
            bcsp = _pool("bcs", 2)
            ysp = _pool("ys", 3)
            sps = _pool("sps", 2, space="PSUM")
            stp = _pool("stp", 2, space="PSUM")
            avp = _pool("avp", 2, space="PSUM")
            # ---- constants -------------------------------------------------
            # latency-tolerant bulk on gpsimd; weights (needed first) on sync
            cos_t = cst.tile([128, T], MD)
            nc.gpsimd.dma_start(out=cos_t, in_=cosd[:, :])
            sin_t = cst.tile([128, T], MD)
            nc.gpsimd.dma_start(out=sin_t, in_=sind[:, :])
            p2_t = cst.tile([128, 128], MD)
            nc.gpsimd.dma_start(out=p2_t, in_=p2d[:, :])
            bandT_t = cst.tile([128, 256], F)  # [band.T | band.T] per head
            nc.gpsimd.dma_start(out=bandT_t, in_=bandTd[:, :])
            idf_t = cst.tile([128, 128], F)
            nc.gpsimd.dma_start(out=idf_t, in_=idfd[:, :])
            idh_t = cst.tile([128, 128], MD)
            nc.gpsimd.dma_start(out=idh_t, in_=idhd[:, :])
            sel_t = cst.tile([1, 64], MD)
            nc.gpsimd.dma_start(out=sel_t, in_=seld[0:1, 0:64])

            wq_t = cst.tile([128, 8, 128], MD)
            wk_t = cst.tile([128, 8, 128], MD)
            wv_t = cst.tile([128, 8, 128], MD)
            for k in range(8):
                nc.sync.dma_start(out=wq_t[:, k, :], in_=wq[k * 128 : (k + 1) * 128, :])
            for k in range(8):
                nc.sync.dma_start(out=wk_t[:, k, :], in_=wk[k * 128 : (k + 1) * 128, :])
            for k in range(8):
                nc.sync.dma_start(out=wv_t[:, k, :], in_=wv[k * 128 : (k + 1) * 128, :])
            wo_t = cst.tile([128, C], MD)
            nc.sync.dma_start(out=wo_t, in_=wo[:, :])

            # ---- filler work queue ----------------------------------------
            fill_q = []
            markers = set()

            def pump_one():
                if not fill_q:
                    return 0
                tag, cost, fn = fill_q.pop(0)
                fn()
                if tag is not None:
                    markers.add(tag)
                return max(cost, 50)

            def pump_ns(budget):
                while fill_q and budget > 0:
                    budget -= pump_one()

            def drain_until(tag):
                while tag not in markers:
                    if not fill_q:
                        raise RuntimeError(f"marker {tag} never enqueued")
                    pump_one()

            QKV = {}  # b -> (Qb, Kb, Vb)

            def alloc_b(b):
                Qb = qp.tile([128, T], MD, name="Qb")
                Kb = kp.tile([128, T], MD, name="Kb")
                Vb = vp.tile([128, 16, 256], MD, name="Vb")  # per head 128 cols:
                # [d 0..63 | ones | zeros*63] so the AV lhsT is 128-wide
                QKV[b] = (Qb, Kb, Vb)
                nc.gpsimd.dma_start(
                    out=Vb[:, :, 64:256:128],
                    in_=vonesd[:, :].rearrange("p (a b) -> p a b", b=2),
                )
                zin = vzerod[:, :].rearrange("p (a b) -> p a b", b=63)
                nc.gpsimd.dma_start(out=Vb[:, :, 65:128], in_=zin)
                nc.gpsimd.dma_start(out=Vb[:, :, 193:256], in_=zin)

            def enqueue_unit(b, nb):
                """Queue the projection/rope/V work for 512-token block nb of
                batch b as fine-grained filler closures."""
                Qb, Kb, Vb = QKV[b]
                g0 = b * T + nb * 512
                cols = slice(nb * 512, (nb + 1) * 512)
                box = {}

                def c_load():
                    xt = xtp.tile([128, 8, 512], MD, name="xt")
                    box["xt"] = xt
                    for k in range(8):
                        nc.sync.dma_start(
                            out=xt[:, k, :],
                            in_=xT[k * 128 : (k + 1) * 128, g0 : g0 + 512],
                        )

                def mk_proj(W, key):
                    def f():
                        ps = sps.tile([128, 512], F, tag="s", name="ps")
                        box[key] = ps
                        xt = box["xt"]
                        for k in range(8):
                            nc.tensor.matmul(
                                ps[:, :], lhsT=W[:, k, :], rhs=xt[:, k, :],
                                start=(k == 0), stop=(k == 7),
                            )
                    return f

                def mk_rope(key, dst):
                    def f():
                        ps = box[key]
                        ev = evp.tile([128, 512], MD, name="ev")
                        nc.scalar.copy(ev[:, :], ps[:, :])
                        qs = qsp.tile([128, 512], MD, name="qs")
                        box[key + "s"] = qs
                        nc.vector.tensor_tensor(qs[:, :], ev[:, :],
                                                sin_t[:, cols], MULT)
                        nc.vector.tensor_tensor(dst[:, cols], ev[:, :],
                                                cos_t[:, cols], MULT)
                    return f

                def mk_rot(key, dst):
                    def f():
                        qs = box[key + "s"]
                        rot = sps.tile([128, 512], F, tag="s", name="rot")
                        nc.tensor.matmul(rot[:, :], lhsT=p2_t[:, :], rhs=qs[:, :],
                                         start=True, stop=True)
                        nc.vector.tensor_tensor(dst[:, cols], dst[:, cols],
                                                rot[:, :], SUB)
                    return f

                def c_vst():
                    ps = box["v"]
                    vst = vstp.tile([128, 512], MD, name="vst")
                    box["vst"] = vst
                    nc.vector.tensor_copy(vst[:, :], ps[:, :])

                def mk_vtr(tls):
                    def f():
                        vst = box["vst"]
                        for tl in tls:
                            tt = nb * 4 + tl
                            tcs = slice(tl * 128, (tl + 1) * 128)
                            # one transpose flips both heads' [d, tok] chunk;
                            # cols 0:64 land in head0's slot, 64:128 head1's
                            tp = sps.tile([128, 128], MD, tag="s", name="tp")
                            nc.tensor.transpose(tp[:, :], vst[:, tcs],
                                                idh_t[:, :])
                            nc.vector.tensor_copy(
                                Vb[:, tt, 0:256].rearrange(
                                    "p (a c) -> p a c", a=2)[:, :, 0:64],
                                tp[:, :].rearrange("p (a c) -> p a c", a=2),
                            )
                    return f

                fill_q.append((None, 0, c_load))
                fill_q.append((None, 900, mk_proj(wq_t, "q")))
                fill_q.append((None, 100, mk_rope("q", Qb)))
                fill_q.append((None, 900, mk_proj(wk_t, "k")))
                fill_q.append((None, 100, mk_rope("k", Kb)))
                fill_q.append((None, 300, mk_rot("q", Qb)))
                fill_q.append((None, 900, mk_proj(wv_t, "v")))
                fill_q.append((None, 300, mk_rot("k", Kb)))
                fill_q.append((None, 100, c_vst))
                fill_q.append((None, 400, mk_vtr((0, 1))))
                fill_q.append((("u", b, nb), 400, mk_vtr((2, 3))))

            def mk_ytt(b, Ob, tt):
                def f():
                    lhs = Ob[:, tt * 128 : (tt + 1) * 128]
                    ysb = ysp.tile([128, 1024], MD, name="ysb")
                    for nh in (0, 1):
                        yp = sps.tile([128, 512], F, tag="s", name="yp")
                        nc.tensor.matmul(
                            yp[:, :], lhsT=lhs,
                            rhs=wo_t[:, nh * 512 : (nh + 1) * 512],
                            start=True, stop=True,
                        )
                        if nh == 0:
                            nc.vector.tensor_copy(ysb[:, 0:512], yp[:, :])
                        else:
                            nc.scalar.copy(ysb[:, 512:1024], yp[:, :])
                    r0 = b * T + tt * 128
                    eng = nc.gpsimd if tt % 2 == 0 else nc.sync
                    eng.dma_start(out=y[r0 : r0 + 128, :], in_=ysb[:, :])
                return f

            def phase_d(b):
                Qb, Kb, Vb = QKV[b]
                Ob = op_.tile([128, T], MD, name="Ob")
                for i in range(4):
                    if b + 1 < B:
                        enqueue_unit(b + 1, i)
                    drain_until(("u", b, i))
                    pump_ns(400)
                    nch = 4 * i + 4
                    av = [avp.tile([128, 512], F, tag="av", name="av")
                          for _ in (0, 1)]
                    sts = {}

                    def emit_st(j):
                        delta = j * 128 - i * 512
                        nl = 512 - max(0, delta)
                        off = 512 - nl
                        st = stp.tile([128, 2, 512], F, name="st")
                        if delta >= 0:
                            # pre-write the causal band via PE transpose (same
                            # queue as the matmuls -> race-free); QK^T then
                            # accumulates on top (start=False) so no mask op
                            # sits between the matmul and the exp
                            for h in (0, 1):
                                nc.tensor.transpose(
                                    st[:, h, 0:128],
                                    bandT_t[:, 128 * h : 128 * h + 128],
                                    idf_t[:, :],
                                )
                        for h in (0, 1):
                            hs = slice(64 * h, 64 * h + 64)
                            kcols = slice(j * 128, (j + 1) * 128)
                            if delta >= 0:
                                q0 = i * 512 + off
                                nc.tensor.matmul(
                                    st[:, h, 0:128],
                                    lhsT=Kb[hs, kcols],
                                    rhs=Qb[hs, q0 : q0 + 128],
                                    start=False, stop=True,
                                    skip_group_check=True,
                                )
                                if nl > 128:
                                    nc.tensor.matmul(
                                        st[:, h, 128:nl],
                                        lhsT=Kb[hs, kcols],
                                        rhs=Qb[hs, q0 + 128 : (i + 1) * 512],
                                        start=True, stop=True,
                                        skip_group_check=True,
                                    )
                            else:
                                nc.tensor.matmul(
                                    st[:, h, 0:512],
                                    lhsT=Kb[hs, kcols],
                                    rhs=Qb[hs, i * 512 : (i + 1) * 512],
                                    start=True, stop=True,
                                    skip_group_check=True,
                                )
                        A = ap_.tile([128, 2, 512], MD, name="A")
                        nc.scalar.activation(
                            A[:, :, 0:nl], st[:, :, 0:nl], EXP, scale=SCALE)
                        sts[j] = (A, off, nl)

                    emit_st(0)
                    for j in range(nch):
                        if j + 1 < nch:
                            emit_st(j + 1)
                        pump_ns(300)
                        A, off, nl = sts.pop(j)
                        for h in (0, 1):
                            nc.tensor.matmul(
                                av[h][0:128, off:512],
                                lhsT=Vb[:, j, 128 * h : 128 * h + 128],
                                rhs=A[:, h, 0:nl],
                                start=(j == 0), stop=(j == nch - 1),
                                skip_group_check=True,
                            )
                    # normalize: evacuate O~+denoms to SBUF (frees PSUM
                    # immediately), repartition the denom rows [1,512] ->
                    # [128,4] via SBUF->SBUF DMA so the DVE reciprocal runs
                    # across partitions (~130ns, not 4us serial), DMA back,
                    # then a deferred closure does the K=1 PE broadcast
                    # matmuls + normalize multiplies so the PE never waits
                    # on this chain. Recip is x16, folded into Wo/16.
                    avs = []
                    for h in (0, 1):
                        a = avsp.tile([65, 512], F, name="avs")
                        nc.vector.tensor_copy(a[:, :], av[h][0:65, :])
                        avs.append(a)
                    drt = drtp.tile([128, 8], F, name="drt")
                    for h in (0, 1):
                        nc.sync.dma_start(out=drt[:, 4 * h : 4 * h + 4],
                                          in_=avs[h][64:65, :])
                    rcp = rcpp.tile([128, 8], F, name="rcp")
                    nc.vector.reciprocal(rcp[:, :], drt[:, :])
                    rc16 = rc16p.tile([128, 8], MD, name="rc16")
                    nc.vector.tensor_scalar(rc16[:, :], rcp[:, :], 16.0,
                                            None, MULT)
                    rrow = []
                    for h in (0, 1):
                        rr = rrp.tile([1, 512], MD, name="rrow")
                        nc.sync.dma_start(out=rr[0:1, :],
                                          in_=rc16[:, 4 * h : 4 * h + 4])
                        rrow.append(rr)

                    def mk_fin(b, Ob, i, avs, rrow):
                        icols = slice(i * 512, (i + 1) * 512)

                        def fin():
                            bc = sps.tile([128, 512], F, tag="s", name="bc")
                            for h in (0, 1):
                                nc.tensor.matmul(
                                    bc[64 * h : 64 * h + 64, :],
                                    lhsT=sel_t[0:1, :], rhs=rrow[h][0:1, :],
                                    start=True, stop=True)
                            for h in (0, 1):
                                bcs = bcsp.tile([64, 512], F, name="bcs")
                                nc.vector.tensor_copy(
                                    bcs[:, :], bc[64 * h : 64 * h + 64, :])
                                nc.vector.tensor_tensor(
                                    Ob[64 * h : 64 * h + 64, icols],
                                    avs[h][0:64, :], bcs[:, :], MULT,
                                )
                            for tt in range(4 * i, 4 * i + 4):
                                fill_q.append((None, 500, mk_ytt(b, Ob, tt)))
                        return fin

                    fill_q.append((None, 600, mk_fin(b, Ob, i, avs, rrow)))

            alloc_b(0)
            for i in range(4):
                enqueue_unit(0, i)
            for b in range(B):
                if b + 1 < B:
                    alloc_b(b + 1)
                phase_d(b)
            while fill_q:
                pump_one()

    _split_sem_waits(nc)
    return nc


# --------------------------------------------------------------------------
def _host_inputs(x, Wq, Wk, Wv):
    """Per-core input dicts (all shared arrays built once)."""
    BF = np.float16
    xT = np.ascontiguousarray(
        np.asarray(x, dtype=np.float32).reshape(BT, C).T).astype(BF)

    # NeoX d-permutation within each head: evens then odds
    dperm = np.concatenate([np.arange(0, D, 2), np.arange(1, D, 2)])

    inv_freq = (1.0 / (10000.0 ** (np.arange(0, D, 2) / D))).astype(np.float64)
    pos = np.arange(T, dtype=np.float64)
    ang = pos[None, :] * inv_freq[:, None]  # (32, T)
    cos32 = np.cos(ang).astype(np.float32)
    sin32 = np.sin(ang).astype(np.float32)
    cos_t = np.tile(np.vstack([cos32, cos32]), (2, 1))  # (128, T)
    sin_t = np.tile(np.vstack([-sin32, sin32]), (2, 1))  # (128, T), sign folded

    p2 = np.zeros((128, 128), dtype=np.float32)
    for hb in (0, 64):
        for i2 in range(32):
            p2[hb + i2, hb + 32 + i2] = 1.0
            p2[hb + 32 + i2, hb + i2] = 1.0

    band = np.where(
        np.arange(128)[None, :] < np.arange(128)[:, None], np.float32(NEG), 0.0
    ).astype(np.float32)
    bandT = np.ascontiguousarray(band.T)
    bandT2x = np.concatenate([bandT, bandT], axis=1)  # (128, 256)
    id128 = np.eye(128, dtype=np.float32)
    sel2 = np.zeros((2, 128), dtype=np.float32)
    sel2[0, 0:64] = 1.0
    sel2[1, 64:128] = 1.0

    Wq = np.asarray(Wq, dtype=np.float32)
    Wk = np.asarray(Wk, dtype=np.float32)
    Wv = np.asarray(Wv, dtype=np.float32)

    in_maps = []
    for c in range(N_CORES):
        sl = slice(128 * c, 128 * (c + 1))
        wq_c = Wq[:, sl].reshape(C, 2, D)[:, :, dperm].reshape(C, 128)
        wk_c = Wk[:, sl].reshape(C, 2, D)[:, :, dperm].reshape(C, 128)
        in_maps.append({
            "xT": xT,
            "wq": np.ascontiguousarray(wq_c).astype(BF),
            "wk": np.ascontiguousarray(wk_c).astype(BF),
            "wv": np.ascontiguousarray(Wv[:, sl]).astype(BF),
            "wo": None,  # set below
            "cos": cos_t.astype(BF),
            "sin2": sin_t.astype(BF),
            "p2": p2.astype(BF),
            "bandT2x": bandT2x,
            "id128f": id128,
            "id128h": id128.astype(BF),
            "sel2": sel2.astype(BF),
            "vones": np.ones((128, 32), dtype=BF),
            "vzero": np.zeros((128, 1008), dtype=BF),
        })
    return in_maps


def kernel(x, Wq, Wk, Wv, Wo, bo):
    global _BUILT, LAST_RESULT
    from concourse.bass_utils import run_bass_kernel_spmd

    if TRACE:
        _install_ntff_hook()

    if _BUILT is None:
        _BUILT = _build()
    nc = _BUILT

    in_maps = _host_inputs(x, Wq, Wk, Wv)
    Wo = np.asarray(Wo, dtype=np.float32)
    for c in range(N_CORES):
        in_maps[c]["wo"] = np.ascontiguousarray(
            Wo[128 * c : 128 * (c + 1), :] / 16.0).astype(np.float16)

    last_err = None
    for attempt in range(3):
        try:
            res = run_bass_kernel_spmd(
                nc, in_maps, core_ids=list(range(N_CORES)), trace=TRACE
            )
            break
        except Exception as e:  # transient NRT device errors: retry
            last_err = e
            import time as _time

            _time.sleep(2.0)
    else:
        raise last_err
    LAST_RESULT = res

    acc = res.results[0]["y"].astype(np.float32)
    for c in range(1, N_CORES):
        acc = acc + res.results[c]["y"].astype(np.float32)
    out = acc + np.asarray(bo, dtype=np.float32)[None, :]
    return out.reshape(B, T, C)
